# revision 63
# baseline (speedup 1.0000x reference)
"""Trainium2 Bass kernel for causal multi-head attention (B=4,T=1024,C=1024,H=16,D=64).

Sharding: 8 cores = 4 batches x 2 query-row parities.  SPMD: every core runs
the identical program; per-core variation is carried in the input data only.

Host-side token permutation: each core receives x[b]^T with columns permuted
to [own-parity tokens | other-parity tokens].  Queries are the first 512
columns, and causality becomes block-triangular: key level j (one 128-key
block per parity half) is attended by queries >= 128j, with a plain
triangular mask on the boundary block.  Masks are host data, so parity never
appears in the program.

Everything flows in bf16 (halves DMA, removes the fp32r small-matmul
penalty); PSUM accumulation stays fp32.

The schedule is built around keeping the PE continuously busy (the tensor
engine clocks down after any idle gap):
  - a short warmup of throwaway matmuls on memset tiles covers the initial
    DMA latency; qT and kT pairs 0-3 then run as two c-outer 8-chain passes
    (chunked, interleaved wq/wk DMAs) so compute rides the weight stream;
    v heads 0-7 follows on the same PSUM tiles;
  - attention starts while the remaining projection chains (kT pairs 4-7,
    v heads 8-15) are fed one piece per head iteration into the attention
    stream, each piece emitted before its first reader;
  - per head: scoresT per key level = kT_blk^T @ qT[128j:] for both parity
    halves into one [128,2,512] PSUM tile; exp on ACT (scale=1/8, bf16 out);
    boundary mask = one multiply per level (DVE/Pool alternating); AV
    accumulates oT[65,512] = [v|1]^T @ attnT; oT is copied to SBUF at once
    so the bank frees early; 1/sumexp is broadcast across 64 rows via a
    DRAM bounce with a stride-0 read (PE rank-1 broadcast for the last 3
    heads to keep the DMA latency off the critical tail);
  - output projection: four row-blocks accumulate concurrently in all 8
    PSUM banks with the last head pair's contribution last, bias added as a
    rank-1 matmul, and the final copies alternate ACT/DVE into halved DMAs.
"""
import sys

sys.path.insert(0, "/opt/trn_rl_repo")
import numpy as np

B, T, C, H, D = 4, 1024, 1024, 16, 64
N_CORES = 8
NCT = C // 128  # 8 contraction tiles
NTT = T // 128  # 8 key blocks (4 levels x 2 parity halves)
NP = H // 2  # 8 head pairs
QR = 512  # query rows per core
NL = 4  # key levels; level j holds key blocks j (own parity) and j+4 (other)
WARMUP = 12  # throwaway 512-row matmuls to keep PE busy until first DMA lands

_CACHE = {}


def _build():
    import concourse.bacc as bacc
    import concourse.mybir as mybir
    import concourse.tile as tile

    F32 = mybir.dt.float32
    BF16 = mybir.dt.bfloat16
    F32R = mybir.dt.float32r
    Exp = mybir.ActivationFunctionType.Exp
    Copy = mybir.ActivationFunctionType.Copy

    def r(ap):
        return ap.bitcast(F32R)

    nc = bacc.Bacc("TRN2", target_bir_lowering=False, debug=False, num_devices=N_CORES)
    xT_d = nc.declare_dram_parameter("xT", [C, T], BF16, isOutput=False)
    wq_d = nc.declare_dram_parameter("wq", [C, H * D], BF16, isOutput=False)
    wk_d = nc.declare_dram_parameter("wk", [C, H * D], BF16, isOutput=False)
    wv_d = nc.declare_dram_parameter("wv", [C, H * D], BF16, isOutput=False)
    woT_d = nc.declare_dram_parameter("woT", [H * D, C], BF16, isOutput=False)
    bo_d = nc.declare_dram_parameter("bo1", [1, C], BF16, isOutput=False)
    m2_d = nc.declare_dram_parameter("m2", [128, 2, 128], BF16, isOutput=False)
    out_d = nc.declare_dram_parameter("out", [QR, C], F32, isOutput=True)
    # DRAM bounce buffer for the per-head 1/sumexp row: written as [1,QR],
    # read back with a stride-0 partition dim to broadcast across 64 rows.
    recscr_d = nc.declare_dram_parameter("recscr", [H, QR], F32, isOutput=True)

    with tile.TileContext(nc) as tc:
        with tc.tile_pool(name="keep", bufs=1) as keep:
            xT = keep.tile([128, NCT, T], BF16)
            qT = keep.tile([128, NP, QR], BF16)
            kT = keep.tile([128, NP, T], BF16)
            vext = keep.tile([128, NTT, H, 65], BF16)
            m2 = keep.tile([128, 2, 128], BF16)
            wlhs = keep.tile([1, 128], BF16)
            ones1f = keep.tile([1, 64], F32)
            ones1s = keep.tile([1, 64], F32)
            wsrc = keep.tile([1, QR], BF16)
            dact = keep.tile([1, 16], F32)
            ones_sb = keep.tile([128, 128], BF16)
            bo1 = keep.tile([1, C], BF16)
            woT = keep.tile([128, NP, C], BF16)
            proj_in = keep.tile([128, NP, QR], BF16)

            # memset-built constants: no DMA dependency, so the PE warmup and
            # the bc broadcast never wait on a transfer.
            nc.vector.memset(wlhs[:], 1.0)
            nc.vector.memset(ones1s[:], 1.0)
            with nc.allow_low_precision(reason="fp32r relabel of fp32 ones"):
                nc.vector.tensor_copy(r(ones1f[:]), ones1s[:])
            nc.gpsimd.memset(wsrc[:], 0.0)
            nc.gpsimd.memset(ones_sb[:], 1.0)
            nc.vector.tensor_copy(
                vext[:, :, :, 64:65], ones_sb[:].rearrange("p (a b) -> p a b", a=NTT))
            # prime the ACT Exp table while the engine is idle
            nc.scalar.activation(dact[:], wsrc[0:1, 0:16], Exp, scale=0.125)

            # SP ring: xT tiles (first-needed, 2 c-tiles per copy), then the
            # mask.  Chunked copies halve the per-DMA overhead.
            for a in range(NCT // 2):
                nc.sync.dma_start(
                    xT[:, 2 * a:2 * a + 2, :],
                    xT_d[2 * a * 128:(2 * a + 2) * 128, :].rearrange(
                        "(a p) t -> p a t", p=128))
            nc.sync.dma_start(m2[:], m2_d[:])
            # ACT ring: weights in consumption order, then phase-3 tiles.
            with tc.tile_pool(name="wp", bufs=1) as wp:
                wq = wp.tile([128, NCT, H * D], BF16)
                wk = wp.tile([128, NCT, H * D], BF16)
                wv = wp.tile([128, NCT, H * D], BF16)
                # wq/wk chunks interleaved so pass 1 (qT-A + kT-u0) can
                # consume both weight streams chunk by chunk
                for a in range(NCT // 2):
                    for w_sb, w_d in ((wq, wq_d), (wk, wk_d)):
                        nc.scalar.dma_start(
                            w_sb[:, 2 * a:2 * a + 2, :],
                            w_d[2 * a * 128:(2 * a + 2) * 128, :].rearrange(
                                "(a p) t -> p a t", p=128))
                for a in range(NCT // 2):
                    nc.scalar.dma_start(
                        wv[:, 2 * a:2 * a + 2, :],
                        wv_d[2 * a * 128:(2 * a + 2) * 128, :].rearrange(
                            "(a p) t -> p a t", p=128))
                for a in range(NP // 2):
                    nc.scalar.dma_start(
                        woT[:, 2 * a:2 * a + 2, :],
                        woT_d[2 * a * 128:(2 * a + 2) * 128, :].rearrange(
                            "(a p) t -> p a t", p=128))
                nc.scalar.dma_start(bo1[:], bo_d[:])

                # ---- phase 1a/1b: PE warmup, then qT and kT pairs 0-3 as
                # c-outer 4-chain passes ping-ponging two PSUM bank groups;
                # each pass's copies drain while the next pass computes. ----
                with tc.tile_pool(name="ps8", bufs=1, space="PSUM") as ps8p:
                    ps8 = [
                        ps8p.tile([128, QR], F32, tag=f"g{i}", name=f"ps8_{i}", bufs=1)
                        for i in range(8)
                    ]
                    for w in range(WARMUP):
                        nc.tensor.matmul(
                            ps8[4 + w % 4][:], wlhs[:], wsrc[:],
                            start=True, stop=True,
                        )

                    def couter_pass(jobs):
                        # jobs: list of (bank, w_sb, pair, cols, dst, copy_act)
                        for c in range(NCT):
                            for i, (w_sb, p, cols, dst, copy_act) in enumerate(jobs):
                                nc.tensor.matmul(
                                    ps8[i][:],
                                    w_sb[:, c, p * 128:(p + 1) * 128],
                                    xT[:, c, cols],
                                    start=(c == 0),
                                    stop=(c == NCT - 1),
                                )
                                if c == NCT - 1:
                                    if copy_act:
                                        nc.scalar.activation(dst, ps8[i][:], Copy)
                                    else:
                                        nc.vector.tensor_copy(dst, ps8[i][:])

                    colsq = slice(0, QR)
                    cols0, cols1 = slice(0, QR), slice(QR, 2 * QR)
                    couter_pass(
                        [(wq, p, colsq, qT[:, p, :], False) for p in range(0, 4)]
                        + [(wk, p, cols0, kT[:, p, cols0], True) for p in range(0, 4)])
                    couter_pass(
                        [(wq, p, colsq, qT[:, p, :], False) for p in range(4, 8)]
                        + [(wk, p, cols1, kT[:, p, cols1], True) for p in range(0, 4)])
                    # vA reuses ps8 group-0 tiles round-robin: no pool
                    # transition, so no wait on the full pool release
                    for tt in range(NTT):
                        psv = ps8[tt % 4]
                        for c in range(NCT):
                            nc.tensor.matmul(
                                psv[:],
                                xT[:, c, tt * 128:(tt + 1) * 128],
                                wv[:, c, 0:QR],
                                start=(c == 0), stop=(c == NCT - 1))
                        nc.vector.tensor_copy(
                            vext[:, tt, 0:8, 0:64],
                            psv[:].rearrange("p (h d) -> p h d", h=8))

                # ---- fused attention + remaining projections ----
                with (
                    tc.tile_pool(name="attn", bufs=8) as attnp,
                    tc.tile_pool(name="smalls", bufs=3) as smalls,
                    tc.tile_pool(name="ps_s", bufs=2, space="PSUM") as ps_s,
                    tc.tile_pool(name="ps_o", bufs=2, space="PSUM") as ps_o,
                    tc.tile_pool(name="ps_x", bufs=2, space="PSUM") as ps_x,
                ):
                    # projection pieces fed one-per-head-iteration into the
                    # attention stream.  vB first (needed by AV of head 8 at
                    # iter 9), kT pairs 4-7 spliced to land before their
                    # scores (head 2p at iter 2p).
                    def vb_piece(tt):
                        def run():
                            psv = ps_x.tile([128, QR], F32, tag="x", name="psvb")
                            for c in range(NCT):
                                nc.tensor.matmul(
                                    psv[:],
                                    xT[:, c, tt * 128:(tt + 1) * 128],
                                    wv[:, c, QR:2 * QR],
                                    start=(c == 0), stop=(c == NCT - 1))
                            nc.vector.tensor_copy(
                                vext[:, tt, 8:16, 0:64],
                                psv[:].rearrange("p (h d) -> p h d", h=8))
                        return run

                    def kt_piece(p, u):
                        def run():
                            psk = ps_x.tile([128, QR], F32, tag="x", name="pskb")
                            for c in range(NCT):
                                nc.tensor.matmul(
                                    psk[:],
                                    wk[:, c, p * 128:(p + 1) * 128],
                                    xT[:, c, u * QR:(u + 1) * QR],
                                    start=(c == 0), stop=(c == NCT - 1))
                            nc.scalar.activation(kT[:, p, u * QR:(u + 1) * QR], psk[:], Copy)
                        return run

                    # per-iteration piece lists.  Deadlines: kT pair p must be
                    # fully written before iteration 2p's scores (slot <=
                    # 2p-1); vB before iteration 9's AV of head 8.
                    pieces = [[] for _ in range(H)]
                    pieces[0] = [vb_piece(0), vb_piece(1)]
                    pieces[1] = [vb_piece(2), vb_piece(3)]
                    pieces[2] = [vb_piece(4)]
                    pieces[3] = [vb_piece(5)]
                    pieces[4] = [vb_piece(6)]
                    pieces[5] = [vb_piece(7)]
                    pieces[6] = [kt_piece(4, 0)]
                    pieces[7] = [kt_piece(4, 1)]
                    pieces[8] = [kt_piece(5, 0)]
                    pieces[9] = [kt_piece(5, 1)]
                    pieces[10] = [kt_piece(6, 0)]
                    pieces[11] = [kt_piece(6, 1)]
                    pieces[12] = [kt_piece(7, 0)]
                    pieces[13] = [kt_piece(7, 1)]

                    tiles = {}  # h -> list of (level, at tile)
                    oTs = {}  # h -> oT psum
                    recbs = {}  # h -> broadcast 1/sumexp tile

                    def stage_s(h, js):
                        p, po = h // 2, (h % 2) * 64
                        lst = tiles.setdefault(h, [])
                        for j in js:
                            st = 128 * j
                            sps = ps_s.tile([128, 2, 512], F32, tag="s")
                            for sub in range(2):
                                kb = j + 4 * sub
                                nc.tensor.matmul(
                                    sps[:, sub, st:],
                                    kT[po:po + 64, p, kb * 128:(kb + 1) * 128],
                                    qT[po:po + 64, p, st:],
                                    start=True,
                                    stop=True,
                                )
                            at = attnp.tile([128, 2, 512], BF16, tag="at")
                            nc.scalar.activation(at[:, :, st:], sps[:, :, st:], Exp, scale=0.125)
                            eng = nc.vector if j % 2 == 0 else nc.gpsimd
                            eng.tensor_mul(
                                at[:, :, st:st + 128], at[:, :, st:st + 128], m2[:])
                            lst.append((j, at))

                    def stage_a(h):
                        oT = ps_o.tile([65, QR], F32, tag="o")
                        for j, at in tiles.pop(h):
                            st = 128 * j
                            for sub in range(2):
                                kb = j + 4 * sub
                                nc.tensor.matmul(
                                    oT[:, st:],
                                    vext[:, kb, h, :],
                                    at[:, sub, st:],
                                    start=(j == 0 and sub == 0),
                                    stop=(j == NL - 1 and sub == 1),
                                    skip_group_check=True,
                                )
                        rec = smalls.tile([1, QR], F32, tag="rec")
                        if h < H - 3:
                            nc.vector.reciprocal(rec[:], oT[64:65, :])
                        else:
                            with nc.allow_low_precision(reason="fp32r relabel of fp32 reciprocal"):
                                nc.vector.reciprocal(r(rec[:]), oT[64:65, :])
                        # copy oT out of PSUM immediately so the bank frees
                        # without waiting for the broadcast round trip
                        oTc = smalls.tile([64, QR], BF16, tag="oTc")
                        with nc.allow_low_precision(reason="attention output rounded to bf16"):
                            nc.vector.tensor_copy(oTc[:], oT[0:64, :])
                        oTs[h] = oTc
                        if h < H - 3:
                            # bounce through DRAM to broadcast across 64 rows
                            nc.sync.dma_start(recscr_d[h:h + 1, :], rec[:])
                            recb = smalls.tile([64, QR], F32, tag="recb")
                            nc.sync.dma_start(
                                recb[:], recscr_d[h:h + 1, :].partition_broadcast(64))
                            recbs[h] = recb
                        else:
                            # tail heads: PE rank-1 broadcast avoids the DMA
                            # round-trip latency on the critical path
                            recbs[h] = rec

                    def stage_n(h):
                        p, po = h // 2, (h % 2) * 64
                        oTc = oTs.pop(h)
                        recb = recbs.pop(h)
                        if h >= H - 3:
                            bcf = ps_x.tile([128, QR], F32, tag="x", name="bc")
                            nc.tensor.matmul(bcf[0:64, :], r(ones1f[:]), r(recb[:]),
                                             start=True, stop=True)
                            recb = bcf[0:64, :]
                        with nc.allow_low_precision(reason="attention output rounded to bf16"):
                            nc.vector.tensor_mul(proj_in[po:po + 64, p, :], oTc[:], recb[:])

                    for h in range(H):
                        stage_s(h, (0, 1))
                        for piece in pieces[h]:
                            piece()
                        stage_s(h, (2, 3))
                        if h >= 1:
                            stage_a(h - 1)
                        if h >= 2:
                            stage_n(h - 2)
                    # tight drain: the last head's AV/normalize gates the
                    # output projection and the final 2MB DMA
                    stage_a(H - 1)
                    stage_n(H - 2)
                    stage_n(H - 1)

            # ---------------- output projection ----------------
            with (
                tc.tile_pool(name="fin", bufs=2) as finp,
                tc.tile_pool(name="ps_f", bufs=1, space="PSUM") as ps_f,
            ):
                # all four row-blocks accumulate concurrently (8 banks); the
                # last head pair's contribution comes last so everything else
                # overlaps the attention drain.
                psfs = [
                    ps_f.tile([128, C], F32, tag=f"f{m}", name=f"psf{m}", bufs=1)
                    for m in range(QR // 128)
                ]
                for p in range(NP - 1):
                    for m in range(QR // 128):
                        psf = psfs[m]
                        lhs = proj_in[:, p, m * 128:(m + 1) * 128]
                        nc.tensor.matmul(psf[:, 0:512], lhs, woT[:, p, 0:512],
                                         start=(p == 0), stop=False,
                                         skip_group_check=True)
                        nc.tensor.matmul(psf[:, 512:1024], lhs, woT[:, p, 512:1024],
                                         start=(p == 0), stop=False,
                                         skip_group_check=True)
                for m in range(QR // 128):
                    psf = psfs[m]
                    lhs = proj_in[:, NP - 1, m * 128:(m + 1) * 128]
                    nc.tensor.matmul(psf[:, 0:512], lhs, woT[:, NP - 1, 0:512],
                                     start=False, stop=False, skip_group_check=True)
                    nc.tensor.matmul(psf[:, 512:1024], lhs, woT[:, NP - 1, 512:1024],
                                     start=False, stop=False, skip_group_check=True)
                    # bias via rank-1 accumulate: psf += 1 (x) bo
                    nc.tensor.matmul(psf[:, 0:512], wlhs[:], bo1[0:1, 0:512],
                                     start=False, stop=True, skip_group_check=True)
                    nc.tensor.matmul(psf[:, 512:1024], wlhs[:], bo1[0:1, 512:1024],
                                     start=False, stop=True, skip_group_check=True)
                    fin = finp.tile([128, C], F32, tag="fin")
                    # halves on alternating engines so the copies drain 2x
                    for u in range(2):
                        half = slice(u * 512, (u + 1) * 512)
                        if u == 0:
                            nc.scalar.activation(fin[:, half], psf[:, half], Copy)
                        else:
                            nc.vector.tensor_copy(fin[:, half], psf[:, half])
                        nc.sync.dma_start(
                            out_d[m * 128:(m + 1) * 128, half], fin[:, half])

    nc.compile()
    return nc


def get_nc():
    if "nc" not in _CACHE:
        _CACHE["nc"] = _build()
    return _CACHE["nc"]


def make_in_maps(x, Wq, Wk, Wv, Wo, bo):
    import ml_dtypes

    BF = ml_dtypes.bfloat16
    x = np.asarray(x, dtype=np.float32)
    wq = np.ascontiguousarray(
        np.asarray(Wq, np.float32).transpose(1, 0, 2).reshape(C, H * D).astype(BF))
    wk = np.ascontiguousarray(
        np.asarray(Wk, np.float32).transpose(1, 0, 2).reshape(C, H * D).astype(BF))
    wv = np.ascontiguousarray(
        np.asarray(Wv, np.float32).transpose(1, 0, 2).reshape(C, H * D).astype(BF))
    woT = np.ascontiguousarray(np.asarray(Wo, np.float32).T.astype(BF))
    bo1 = np.ascontiguousarray(np.asarray(bo, np.float32).reshape(1, C).astype(BF))
    k_ = np.arange(128)[:, None]
    i_ = np.arange(128)[None, :]
    m_own = (k_ <= i_).astype(BF)  # own-parity boundary block: k <= q
    in_maps = []
    for core in range(N_CORES):
        b, par = core // 2, core % 2
        perm = np.concatenate([np.arange(par, T, 2), np.arange(1 - par, T, 2)])
        xTp = np.ascontiguousarray(x[b].T[:, perm].astype(BF))
        # other-parity keys 2k+(1-par) vs queries 2i+par: strict for par=0
        m_oth = ((k_ < i_) if par == 0 else (k_ <= i_)).astype(BF)
        m2 = np.ascontiguousarray(np.stack([m_own, m_oth], axis=1))
        in_maps.append({
            "xT": xTp, "wq": wq, "wk": wk, "wv": wv,
            "woT": woT, "bo1": bo1, "m2": m2,
        })
    return in_maps


def kernel(x, Wq, Wk, Wv, Wo, bo):
    from concourse.bass_utils import run_bass_kernel_spmd

    nc = get_nc()
    in_maps = make_in_maps(x, Wq, Wk, Wv, Wo, bo)
    res = run_bass_kernel_spmd(nc, in_maps, list(range(N_CORES)))
    out = np.empty((B, T, C), np.float32)
    for core in range(N_CORES):
        b, par = core // 2, core % 2
        out[b, par::2, :] = res.results[core]["out"]
    return out


# revision 70
# speedup vs baseline: 1.0464x; 1.0464x over previous
"""Trainium2 Bass kernel for causal multi-head attention (B=4,T=1024,C=1024,H=16,D=64).

Sharding: 8 cores = 4 batches x 2 query-row parities.  SPMD: every core runs
the identical program; per-core variation is carried in the input data only.

Host-side token permutation: each core receives x[b]^T with columns permuted
to [own-parity tokens | other-parity tokens].  Queries are the first 512
columns, and causality becomes block-triangular: key level j (one 128-key
block per parity half) is attended by queries >= 128j, with a plain
triangular mask on the boundary block.  Masks are host data, so parity never
appears in the program.

Everything flows in bf16 (halves DMA, removes the fp32r small-matmul
penalty); PSUM accumulation stays fp32.

The schedule is built around keeping the PE continuously busy (the tensor
engine clocks down after any idle gap):
  - a short warmup of throwaway matmuls on memset tiles covers the initial
    DMA latency; qT and kT pairs 0-3 then run as two c-outer 8-chain passes
    (chunked, interleaved wq/wk DMAs) so compute rides the weight stream;
    v heads 0-7 follows on the same PSUM tiles;
  - attention starts while the remaining projection chains (kT pairs 4-7,
    v heads 8-15) are fed one piece per head iteration into the attention
    stream, each piece emitted before its first reader;
  - per head: scoresT per key level = kT_blk^T @ qT[128j:] for both parity
    halves into one [128,2,512] PSUM tile; exp on ACT (scale=1/8, bf16 out);
    boundary mask = one multiply per level (DVE/Pool alternating); AV
    accumulates oT[65,512] = [v|1]^T @ attnT; oT is copied to SBUF at once
    so the bank frees early; 1/sumexp is broadcast across 64 rows via a
    DRAM bounce with a stride-0 read (PE rank-1 broadcast for the last 3
    heads to keep the DMA latency off the critical tail);
  - output projection: row-blocks finalize serially (the last normalize
    lands during the attention drain), so each block's halved ACT/DVE copies
    and output DMAs hide under the next block's matmul chain; bias is added
    as a rank-1 matmul.
"""
import sys

sys.path.insert(0, "/opt/trn_rl_repo")
import numpy as np

B, T, C, H, D = 4, 1024, 1024, 16, 64
N_CORES = 8
NCT = C // 128  # 8 contraction tiles
NTT = T // 128  # 8 key blocks (4 levels x 2 parity halves)
NP = H // 2  # 8 head pairs
QR = 512  # query rows per core
NL = 4  # key levels; level j holds key blocks j (own parity) and j+4 (other)
WARMUP = 12  # throwaway 512-row matmuls to keep PE busy until first DMA lands

_CACHE = {}


def _build():
    import concourse.bacc as bacc
    import concourse.mybir as mybir
    import concourse.tile as tile

    F32 = mybir.dt.float32
    BF16 = mybir.dt.bfloat16
    F32R = mybir.dt.float32r
    Exp = mybir.ActivationFunctionType.Exp
    Copy = mybir.ActivationFunctionType.Copy

    def r(ap):
        return ap.bitcast(F32R)

    nc = bacc.Bacc("TRN2", target_bir_lowering=False, debug=False, num_devices=N_CORES)
    xT_d = nc.declare_dram_parameter("xT", [C, T], BF16, isOutput=False)
    wq_d = nc.declare_dram_parameter("wq", [C, H * D], BF16, isOutput=False)
    wk_d = nc.declare_dram_parameter("wk", [C, H * D], BF16, isOutput=False)
    wv_d = nc.declare_dram_parameter("wv", [C, H * D], BF16, isOutput=False)
    woT_d = nc.declare_dram_parameter("woT", [H * D, C], BF16, isOutput=False)
    bo_d = nc.declare_dram_parameter("bo1", [1, C], BF16, isOutput=False)
    m2_d = nc.declare_dram_parameter("m2", [128, 2, 128], BF16, isOutput=False)
    out_d = nc.declare_dram_parameter("out", [QR, C], F32, isOutput=True)
    # DRAM bounce buffer for the per-head 1/sumexp row: written as [1,QR],
    # read back with a stride-0 partition dim to broadcast across 64 rows.
    recscr_d = nc.declare_dram_parameter("recscr", [H, QR], F32, isOutput=True)

    with tile.TileContext(nc) as tc:
        with tc.tile_pool(name="keep", bufs=1) as keep:
            xT = keep.tile([128, NCT, T], BF16)
            qT = keep.tile([128, NP, QR], BF16)
            kT = keep.tile([128, NP, T], BF16)
            vext = keep.tile([128, NTT, H, 65], BF16)
            m2 = keep.tile([128, 2, 128], BF16)
            wlhs = keep.tile([1, 128], BF16)
            ones1f = keep.tile([1, 64], F32)
            ones1s = keep.tile([1, 64], F32)
            wsrc = keep.tile([1, QR], BF16)
            dact = keep.tile([1, 16], F32)
            ones_sb = keep.tile([128, 128], BF16)
            bo1 = keep.tile([1, C], BF16)
            woT = keep.tile([128, NP, C], BF16)
            proj_in = keep.tile([128, NP, QR], BF16)

            # memset-built constants: no DMA dependency, so the PE warmup and
            # the bc broadcast never wait on a transfer.
            nc.vector.memset(wlhs[:], 1.0)
            nc.vector.memset(ones1s[:], 1.0)
            with nc.allow_low_precision(reason="fp32r relabel of fp32 ones"):
                nc.vector.tensor_copy(r(ones1f[:]), ones1s[:])
            nc.gpsimd.memset(wsrc[:], 0.0)
            nc.gpsimd.memset(ones_sb[:], 1.0)
            nc.vector.tensor_copy(
                vext[:, :, :, 64:65], ones_sb[:].rearrange("p (a b) -> p a b", a=NTT))
            # prime the ACT Exp table while the engine is idle
            nc.scalar.activation(dact[:], wsrc[0:1, 0:16], Exp, scale=0.125)

            # SP ring: xT tiles (first-needed, 2 c-tiles per copy), then the
            # mask.  Chunked copies halve the per-DMA overhead.
            for a in range(NCT // 2):
                nc.sync.dma_start(
                    xT[:, 2 * a:2 * a + 2, :],
                    xT_d[2 * a * 128:(2 * a + 2) * 128, :].rearrange(
                        "(a p) t -> p a t", p=128))
            nc.sync.dma_start(m2[:], m2_d[:])
            # ACT ring: weights in consumption order, then phase-3 tiles.
            with tc.tile_pool(name="wp", bufs=1) as wp:
                wq = wp.tile([128, NCT, H * D], BF16)
                wk = wp.tile([128, NCT, H * D], BF16)
                wv = wp.tile([128, NCT, H * D], BF16)
                # wq/wk chunks interleaved so pass 1 (qT-A + kT-u0) can
                # consume both weight streams chunk by chunk
                for a in range(NCT // 2):
                    for w_sb, w_d in ((wq, wq_d), (wk, wk_d)):
                        nc.scalar.dma_start(
                            w_sb[:, 2 * a:2 * a + 2, :],
                            w_d[2 * a * 128:(2 * a + 2) * 128, :].rearrange(
                                "(a p) t -> p a t", p=128))
                for a in range(NCT // 2):
                    nc.scalar.dma_start(
                        wv[:, 2 * a:2 * a + 2, :],
                        wv_d[2 * a * 128:(2 * a + 2) * 128, :].rearrange(
                            "(a p) t -> p a t", p=128))
                for a in range(NP // 2):
                    nc.scalar.dma_start(
                        woT[:, 2 * a:2 * a + 2, :],
                        woT_d[2 * a * 128:(2 * a + 2) * 128, :].rearrange(
                            "(a p) t -> p a t", p=128))
                nc.scalar.dma_start(bo1[:], bo_d[:])

                # ---- phase 1a/1b: PE warmup, then qT and kT pairs 0-3 as
                # c-outer 4-chain passes ping-ponging two PSUM bank groups;
                # each pass's copies drain while the next pass computes. ----
                with tc.tile_pool(name="ps8", bufs=1, space="PSUM") as ps8p:
                    ps8 = [
                        ps8p.tile([128, QR], F32, tag=f"g{i}", name=f"ps8_{i}", bufs=1)
                        for i in range(8)
                    ]
                    for w in range(WARMUP):
                        nc.tensor.matmul(
                            ps8[4 + w % 4][:], wlhs[:], wsrc[:],
                            start=True, stop=True,
                        )

                    def couter_pass(jobs):
                        # jobs: list of (bank, w_sb, pair, cols, dst, copy_act)
                        for c in range(NCT):
                            for i, (w_sb, p, cols, dst, copy_act) in enumerate(jobs):
                                nc.tensor.matmul(
                                    ps8[i][:],
                                    w_sb[:, c, p * 128:(p + 1) * 128],
                                    xT[:, c, cols],
                                    start=(c == 0),
                                    stop=(c == NCT - 1),
                                )
                                if c == NCT - 1:
                                    if copy_act:
                                        nc.scalar.activation(dst, ps8[i][:], Copy)
                                    else:
                                        nc.vector.tensor_copy(dst, ps8[i][:])

                    colsq = slice(0, QR)
                    cols0, cols1 = slice(0, QR), slice(QR, 2 * QR)
                    couter_pass(
                        [(wq, p, colsq, qT[:, p, :], False) for p in range(0, 4)]
                        + [(wk, p, cols0, kT[:, p, cols0], True) for p in range(0, 4)])
                    couter_pass(
                        [(wq, p, colsq, qT[:, p, :], False) for p in range(4, 8)]
                        + [(wk, p, cols1, kT[:, p, cols1], True) for p in range(0, 4)])
                    # vA reuses ps8 group-0 tiles round-robin: no pool
                    # transition, so no wait on the full pool release
                    for tt in range(NTT):
                        psv = ps8[tt % 4]
                        for c in range(NCT):
                            nc.tensor.matmul(
                                psv[:],
                                xT[:, c, tt * 128:(tt + 1) * 128],
                                wv[:, c, 0:QR],
                                start=(c == 0), stop=(c == NCT - 1))
                        nc.vector.tensor_copy(
                            vext[:, tt, 0:8, 0:64],
                            psv[:].rearrange("p (h d) -> p h d", h=8))

                # ---- fused attention + remaining projections ----
                with (
                    tc.tile_pool(name="attn", bufs=8) as attnp,
                    tc.tile_pool(name="smalls", bufs=3) as smalls,
                    tc.tile_pool(name="ps_s", bufs=2, space="PSUM") as ps_s,
                    tc.tile_pool(name="ps_o", bufs=2, space="PSUM") as ps_o,
                    tc.tile_pool(name="ps_x", bufs=2, space="PSUM") as ps_x,
                ):
                    # projection pieces fed one-per-head-iteration into the
                    # attention stream.  vB first (needed by AV of head 8 at
                    # iter 9), kT pairs 4-7 spliced to land before their
                    # scores (head 2p at iter 2p).
                    def vb_piece(tt):
                        def run():
                            psv = ps_x.tile([128, QR], F32, tag="x", name="psvb")
                            for c in range(NCT):
                                nc.tensor.matmul(
                                    psv[:],
                                    xT[:, c, tt * 128:(tt + 1) * 128],
                                    wv[:, c, QR:2 * QR],
                                    start=(c == 0), stop=(c == NCT - 1))
                            nc.vector.tensor_copy(
                                vext[:, tt, 8:16, 0:64],
                                psv[:].rearrange("p (h d) -> p h d", h=8))
                        return run

                    def kt_piece(p, u):
                        def run():
                            psk = ps_x.tile([128, QR], F32, tag="x", name="pskb")
                            for c in range(NCT):
                                nc.tensor.matmul(
                                    psk[:],
                                    wk[:, c, p * 128:(p + 1) * 128],
                                    xT[:, c, u * QR:(u + 1) * QR],
                                    start=(c == 0), stop=(c == NCT - 1))
                            nc.scalar.activation(kT[:, p, u * QR:(u + 1) * QR], psk[:], Copy)
                        return run

                    # per-iteration piece lists.  Deadlines: kT pair p must be
                    # fully written before iteration 2p's scores (slot <=
                    # 2p-1); vB before iteration 9's AV of head 8.
                    pieces = [[] for _ in range(H)]
                    pieces[0] = [vb_piece(0), vb_piece(1)]
                    pieces[1] = [vb_piece(2), vb_piece(3)]
                    pieces[2] = [vb_piece(4)]
                    pieces[3] = [vb_piece(5)]
                    pieces[4] = [vb_piece(6)]
                    pieces[5] = [vb_piece(7)]
                    pieces[6] = [kt_piece(4, 0)]
                    pieces[7] = [kt_piece(4, 1)]
                    pieces[8] = [kt_piece(5, 0)]
                    pieces[9] = [kt_piece(5, 1)]
                    pieces[10] = [kt_piece(6, 0)]
                    pieces[11] = [kt_piece(6, 1)]
                    pieces[12] = [kt_piece(7, 0)]
                    pieces[13] = [kt_piece(7, 1)]

                    tiles = {}  # h -> list of (level, at tile)
                    oTs = {}  # h -> oT psum
                    recbs = {}  # h -> broadcast 1/sumexp tile

                    def stage_s(h, js):
                        p, po = h // 2, (h % 2) * 64
                        lst = tiles.setdefault(h, [])
                        for j in js:
                            st = 128 * j
                            sps = ps_s.tile([128, 2, 512], F32, tag="s")
                            for sub in range(2):
                                kb = j + 4 * sub
                                nc.tensor.matmul(
                                    sps[:, sub, st:],
                                    kT[po:po + 64, p, kb * 128:(kb + 1) * 128],
                                    qT[po:po + 64, p, st:],
                                    start=True,
                                    stop=True,
                                )
                            at = attnp.tile([128, 2, 512], BF16, tag="at")
                            nc.scalar.activation(at[:, :, st:], sps[:, :, st:], Exp, scale=0.125)
                            eng = nc.vector if j % 2 == 0 else nc.gpsimd
                            eng.tensor_mul(
                                at[:, :, st:st + 128], at[:, :, st:st + 128], m2[:])
                            lst.append((j, at))

                    def stage_a(h):
                        oT = ps_o.tile([65, QR], F32, tag="o")
                        for j, at in tiles.pop(h):
                            st = 128 * j
                            for sub in range(2):
                                kb = j + 4 * sub
                                nc.tensor.matmul(
                                    oT[:, st:],
                                    vext[:, kb, h, :],
                                    at[:, sub, st:],
                                    start=(j == 0 and sub == 0),
                                    stop=(j == NL - 1 and sub == 1),
                                    skip_group_check=True,
                                )
                        rec = smalls.tile([1, QR], F32, tag="rec")
                        if h < H - 3:
                            nc.vector.reciprocal(rec[:], oT[64:65, :])
                        else:
                            with nc.allow_low_precision(reason="fp32r relabel of fp32 reciprocal"):
                                nc.vector.reciprocal(r(rec[:]), oT[64:65, :])
                        # copy oT out of PSUM immediately so the bank frees
                        # without waiting for the broadcast round trip
                        oTc = smalls.tile([64, QR], BF16, tag="oTc")
                        with nc.allow_low_precision(reason="attention output rounded to bf16"):
                            nc.vector.tensor_copy(oTc[:], oT[0:64, :])
                        oTs[h] = oTc
                        if h < H - 3:
                            # bounce through DRAM to broadcast across 64 rows
                            nc.sync.dma_start(recscr_d[h:h + 1, :], rec[:])
                            recb = smalls.tile([64, QR], F32, tag="recb")
                            nc.sync.dma_start(
                                recb[:], recscr_d[h:h + 1, :].partition_broadcast(64))
                            recbs[h] = recb
                        else:
                            # tail heads: PE rank-1 broadcast avoids the DMA
                            # round-trip latency on the critical path
                            recbs[h] = rec

                    def stage_n(h):
                        p, po = h // 2, (h % 2) * 64
                        oTc = oTs.pop(h)
                        recb = recbs.pop(h)
                        if h >= H - 3:
                            bcf = ps_x.tile([128, QR], F32, tag="x", name="bc")
                            nc.tensor.matmul(bcf[0:64, :], r(ones1f[:]), r(recb[:]),
                                             start=True, stop=True)
                            recb = bcf[0:64, :]
                        with nc.allow_low_precision(reason="attention output rounded to bf16"):
                            nc.vector.tensor_mul(proj_in[po:po + 64, p, :], oTc[:], recb[:])

                    for h in range(H):
                        stage_s(h, (0, 1))
                        for piece in pieces[h]:
                            piece()
                        stage_s(h, (2, 3))
                        if h >= 1:
                            stage_a(h - 1)
                        if h >= 2:
                            stage_n(h - 2)
                    # tight drain: the last head's AV/normalize gates the
                    # output projection and the final 2MB DMA
                    stage_a(H - 1)
                    stage_n(H - 2)
                    stage_n(H - 1)

            # ---------------- output projection ----------------
            with (
                tc.tile_pool(name="fin", bufs=2) as finp,
                tc.tile_pool(name="ps_f", bufs=2, space="PSUM") as ps_f,
            ):
                # row-blocks finalize serially: mul(15) already landed during
                # the attention drain, so each block's output DMA hides under
                # the next block's matmul chain; only m3's DMA is exposed.
                for m in range(QR // 128):
                    psf = ps_f.tile([128, C], F32, tag="f")
                    for p in range(NP):
                        lhs = proj_in[:, p, m * 128:(m + 1) * 128]
                        nc.tensor.matmul(psf[:, 0:512], lhs, woT[:, p, 0:512],
                                         start=(p == 0), stop=False,
                                         skip_group_check=True)
                        nc.tensor.matmul(psf[:, 512:1024], lhs, woT[:, p, 512:1024],
                                         start=(p == 0), stop=False,
                                         skip_group_check=True)
                    # bias via rank-1 accumulate: psf += 1 (x) bo
                    nc.tensor.matmul(psf[:, 0:512], wlhs[:], bo1[0:1, 0:512],
                                     start=False, stop=True, skip_group_check=True)
                    nc.tensor.matmul(psf[:, 512:1024], wlhs[:], bo1[0:1, 512:1024],
                                     start=False, stop=True, skip_group_check=True)
                    fin = finp.tile([128, C], F32, tag="fin")
                    # halves on alternating engines so the copies drain 2x
                    for u in range(2):
                        half = slice(u * 512, (u + 1) * 512)
                        if u == 0:
                            nc.scalar.activation(fin[:, half], psf[:, half], Copy)
                        else:
                            nc.vector.tensor_copy(fin[:, half], psf[:, half])
                        nc.sync.dma_start(
                            out_d[m * 128:(m + 1) * 128, half], fin[:, half])

    nc.compile()
    return nc


def get_nc():
    if "nc" not in _CACHE:
        _CACHE["nc"] = _build()
    return _CACHE["nc"]


def make_in_maps(x, Wq, Wk, Wv, Wo, bo):
    import ml_dtypes

    BF = ml_dtypes.bfloat16
    x = np.asarray(x, dtype=np.float32)
    wq = np.ascontiguousarray(
        np.asarray(Wq, np.float32).transpose(1, 0, 2).reshape(C, H * D).astype(BF))
    wk = np.ascontiguousarray(
        np.asarray(Wk, np.float32).transpose(1, 0, 2).reshape(C, H * D).astype(BF))
    wv = np.ascontiguousarray(
        np.asarray(Wv, np.float32).transpose(1, 0, 2).reshape(C, H * D).astype(BF))
    woT = np.ascontiguousarray(np.asarray(Wo, np.float32).T.astype(BF))
    bo1 = np.ascontiguousarray(np.asarray(bo, np.float32).reshape(1, C).astype(BF))
    k_ = np.arange(128)[:, None]
    i_ = np.arange(128)[None, :]
    m_own = (k_ <= i_).astype(BF)  # own-parity boundary block: k <= q
    in_maps = []
    for core in range(N_CORES):
        b, par = core // 2, core % 2
        perm = np.concatenate([np.arange(par, T, 2), np.arange(1 - par, T, 2)])
        xTp = np.ascontiguousarray(x[b].T[:, perm].astype(BF))
        # other-parity keys 2k+(1-par) vs queries 2i+par: strict for par=0
        m_oth = ((k_ < i_) if par == 0 else (k_ <= i_)).astype(BF)
        m2 = np.ascontiguousarray(np.stack([m_own, m_oth], axis=1))
        in_maps.append({
            "xT": xTp, "wq": wq, "wk": wk, "wv": wv,
            "woT": woT, "bo1": bo1, "m2": m2,
        })
    return in_maps


def kernel(x, Wq, Wk, Wv, Wo, bo):
    from concourse.bass_utils import run_bass_kernel_spmd

    nc = get_nc()
    in_maps = make_in_maps(x, Wq, Wk, Wv, Wo, bo)
    res = run_bass_kernel_spmd(nc, in_maps, list(range(N_CORES)))
    out = np.empty((B, T, C), np.float32)
    for core in range(N_CORES):
        b, par = core // 2, core % 2
        out[b, par::2, :] = res.results[core]["out"]
    return out


# revision 77
# speedup vs baseline: 1.0640x; 1.0167x over previous
"""Trainium2 Bass kernel for causal multi-head attention (B=4,T=1024,C=1024,H=16,D=64).

Sharding: 8 cores = 4 batches x 2 query-row parities.  SPMD: every core runs
the identical program; per-core variation is carried in the input data only.

Host-side token permutation: each core receives x[b]^T with columns permuted
to [own-parity tokens | other-parity tokens].  Queries are the first 512
columns, and causality becomes block-triangular: key level j (one 128-key
block per parity half) is attended by queries >= 128j, with a plain
triangular mask on the boundary block.  Masks are host data, so parity never
appears in the program.

Everything flows in bf16 (halves DMA, removes the fp32r small-matmul
penalty); PSUM accumulation stays fp32.

The schedule is built around keeping the PE continuously busy (the tensor
engine clocks down after any idle gap):
  - a short warmup of throwaway matmuls on memset tiles covers the initial
    DMA latency; qT and kT pairs 0-3 then run as two c-outer 8-chain passes
    (chunked, interleaved wq/wk DMAs) so compute rides the weight stream;
    v heads 0-7 follows on the same PSUM tiles;
  - attention starts while the remaining projection chains (kT pairs 4-7,
    v heads 8-15) are fed one piece per head iteration into the attention
    stream, each piece emitted before its first reader;
  - per head: scoresT per key level = kT_blk^T @ qT[128j:] for both parity
    halves into one [128,2,512] PSUM tile; exp on ACT (scale=1/8, bf16 out);
    boundary mask = one multiply per level (DVE/Pool alternating); AV
    accumulates oT[65,512] = [v|1]^T @ attnT; oT is copied to SBUF at once
    so the bank frees early; 1/sumexp is broadcast across 64 rows via a
    DRAM bounce with a stride-0 read (PE rank-1 broadcast for the last 3
    heads to keep the DMA latency off the critical tail);
  - output projection: row-blocks finalize serially (the last normalize
    lands during the attention drain), so each block's halved ACT/DVE copies
    and output DMAs hide under the next block's matmul chain; bias is added
    as a rank-1 matmul.
"""
import sys

sys.path.insert(0, "/opt/trn_rl_repo")
import numpy as np

B, T, C, H, D = 4, 1024, 1024, 16, 64
N_CORES = 8
NCT = C // 128  # 8 contraction tiles
NTT = T // 128  # 8 key blocks (4 levels x 2 parity halves)
NP = H // 2  # 8 head pairs
QR = 512  # query rows per core
NL = 4  # key levels; level j holds key blocks j (own parity) and j+4 (other)
WARMUP = 12  # throwaway 512-row matmuls to keep PE busy until first DMA lands

_CACHE = {}


def _build():
    import concourse.bacc as bacc
    import concourse.mybir as mybir
    import concourse.tile as tile

    F32 = mybir.dt.float32
    BF16 = mybir.dt.bfloat16
    F32R = mybir.dt.float32r
    Exp = mybir.ActivationFunctionType.Exp
    Copy = mybir.ActivationFunctionType.Copy

    def r(ap):
        return ap.bitcast(F32R)

    nc = bacc.Bacc("TRN2", target_bir_lowering=False, debug=False, num_devices=N_CORES)
    xT_d = nc.declare_dram_parameter("xT", [C, T], BF16, isOutput=False)
    # weights pre-split into column halves on the host so every DMA is a
    # contiguous block; pass 1 only needs the A halves of wq/wk
    wqA_d = nc.declare_dram_parameter("wqA", [C, QR], BF16, isOutput=False)
    wqB_d = nc.declare_dram_parameter("wqB", [C, QR], BF16, isOutput=False)
    wkA_d = nc.declare_dram_parameter("wkA", [C, QR], BF16, isOutput=False)
    wkB_d = nc.declare_dram_parameter("wkB", [C, QR], BF16, isOutput=False)
    wvA_d = nc.declare_dram_parameter("wvA", [C, QR], BF16, isOutput=False)
    wvB_d = nc.declare_dram_parameter("wvB", [C, QR], BF16, isOutput=False)
    woT_d = nc.declare_dram_parameter("woT", [H * D, C], BF16, isOutput=False)
    bo_d = nc.declare_dram_parameter("bo1", [1, C], BF16, isOutput=False)
    m2_d = nc.declare_dram_parameter("m2", [128, 2, 128], BF16, isOutput=False)
    out_d = nc.declare_dram_parameter("out", [QR, C], F32, isOutput=True)
    # DRAM bounce buffer for the per-head 1/sumexp row: written as [1,QR],
    # read back with a stride-0 partition dim to broadcast across 64 rows.
    recscr_d = nc.declare_dram_parameter("recscr", [H, QR], F32, isOutput=True)

    with tile.TileContext(nc) as tc:
        with tc.tile_pool(name="keep", bufs=1) as keep:
            xT = keep.tile([128, NCT, T], BF16)
            qT = keep.tile([128, NP, QR], BF16)
            kT = keep.tile([128, NP, T], BF16)
            vext = keep.tile([128, NTT, H, 65], BF16)
            m2 = keep.tile([128, 2, 128], BF16)
            wlhs = keep.tile([1, 128], BF16)
            ones1f = keep.tile([1, 64], F32)
            ones1s = keep.tile([1, 64], F32)
            wsrc = keep.tile([1, QR], BF16)
            dact = keep.tile([1, 16], F32)
            ones_sb = keep.tile([128, 128], BF16)
            bo1 = keep.tile([1, C], BF16)
            woT = keep.tile([128, NP, C], BF16)
            proj_in = keep.tile([128, NP, QR], BF16)

            # memset-built constants: no DMA dependency, so the PE warmup and
            # the bc broadcast never wait on a transfer.
            nc.vector.memset(wlhs[:], 1.0)
            nc.vector.memset(ones1s[:], 1.0)
            with nc.allow_low_precision(reason="fp32r relabel of fp32 ones"):
                nc.vector.tensor_copy(r(ones1f[:]), ones1s[:])
            nc.gpsimd.memset(wsrc[:], 0.0)
            nc.gpsimd.memset(ones_sb[:], 1.0)
            nc.vector.tensor_copy(
                vext[:, :, :, 64:65], ones_sb[:].rearrange("p (a b) -> p a b", a=NTT))
            # prime the ACT Exp table while the engine is idle
            nc.scalar.activation(dact[:], wsrc[0:1, 0:16], Exp, scale=0.125)

            # SP ring: xT tiles (first-needed, 2 c-tiles per copy), then the
            # mask.  Chunked copies halve the per-DMA overhead.
            for a in range(NCT // 2):
                nc.sync.dma_start(
                    xT[:, 2 * a:2 * a + 2, :],
                    xT_d[2 * a * 128:(2 * a + 2) * 128, :].rearrange(
                        "(a p) t -> p a t", p=128))
            nc.sync.dma_start(m2[:], m2_d[:])
            # ACT ring: weights in consumption order, then phase-3 tiles.
            with tc.tile_pool(name="wp", bufs=1) as wp:
                wqA = wp.tile([128, NCT, QR], BF16)
                wqB = wp.tile([128, NCT, QR], BF16)
                wkA = wp.tile([128, NCT, QR], BF16)
                wkB = wp.tile([128, NCT, QR], BF16)
                wvA = wp.tile([128, NCT, QR], BF16)
                wvB = wp.tile([128, NCT, QR], BF16)

                def w_chunk(w_sb, w_d, a):
                    nc.scalar.dma_start(
                        w_sb[:, 2 * a:2 * a + 2, :],
                        w_d[2 * a * 128:(2 * a + 2) * 128, :].rearrange(
                            "(a p) t -> p a t", p=128))

                # pass 1 consumes only the A halves of wq/wk; stream those
                # first, interleaved chunk by chunk
                for a in range(NCT // 2):
                    w_chunk(wqA, wqA_d, a)
                    w_chunk(wkA, wkA_d, a)
                for a in range(NCT // 2):
                    w_chunk(wqB, wqB_d, a)
                for a in range(NCT // 2):
                    w_chunk(wvA, wvA_d, a)
                for a in range(NCT // 2):
                    w_chunk(wkB, wkB_d, a)
                for a in range(NCT // 2):
                    w_chunk(wvB, wvB_d, a)
                for a in range(NP // 2):
                    nc.scalar.dma_start(
                        woT[:, 2 * a:2 * a + 2, :],
                        woT_d[2 * a * 128:(2 * a + 2) * 128, :].rearrange(
                            "(a p) t -> p a t", p=128))
                nc.scalar.dma_start(bo1[:], bo_d[:])

                # ---- phase 1a/1b: PE warmup, then qT and kT pairs 0-3 as
                # c-outer 4-chain passes ping-ponging two PSUM bank groups;
                # each pass's copies drain while the next pass computes. ----
                with tc.tile_pool(name="ps8", bufs=1, space="PSUM") as ps8p:
                    ps8 = [
                        ps8p.tile([128, QR], F32, tag=f"g{i}", name=f"ps8_{i}", bufs=1)
                        for i in range(8)
                    ]
                    for w in range(WARMUP):
                        nc.tensor.matmul(
                            ps8[4 + w % 4][:], wlhs[:], wsrc[:],
                            start=True, stop=True,
                        )

                    def couter_pass(jobs):
                        # jobs: list of (bank, w_sb, pair, cols, dst, copy_act)
                        for c in range(NCT):
                            for i, (w_sb, p, cols, dst, copy_act) in enumerate(jobs):
                                nc.tensor.matmul(
                                    ps8[i][:],
                                    w_sb[:, c, p * 128:(p + 1) * 128],
                                    xT[:, c, cols],
                                    start=(c == 0),
                                    stop=(c == NCT - 1),
                                )
                                if c == NCT - 1:
                                    if copy_act:
                                        nc.scalar.activation(dst, ps8[i][:], Copy)
                                    else:
                                        nc.vector.tensor_copy(dst, ps8[i][:])

                    colsq = slice(0, QR)
                    cols0, cols1 = slice(0, QR), slice(QR, 2 * QR)
                    couter_pass(
                        [(wqA, p, colsq, qT[:, p, :], False) for p in range(0, 4)]
                        + [(wkA, p, cols0, kT[:, p, cols0], True) for p in range(0, 4)])
                    couter_pass(
                        [(wqB, p - 4, colsq, qT[:, p, :], False) for p in range(4, 8)]
                        + [(wkA, p, cols1, kT[:, p, cols1], True) for p in range(0, 4)])
                    # vA reuses ps8 group-0 tiles round-robin: no pool
                    # transition, so no wait on the full pool release
                    for tt in range(NTT):
                        psv = ps8[tt % 4]
                        for c in range(NCT):
                            nc.tensor.matmul(
                                psv[:],
                                xT[:, c, tt * 128:(tt + 1) * 128],
                                wvA[:, c, :],
                                start=(c == 0), stop=(c == NCT - 1))
                        nc.vector.tensor_copy(
                            vext[:, tt, 0:8, 0:64],
                            psv[:].rearrange("p (h d) -> p h d", h=8))

                # ---- fused attention + remaining projections ----
                with (
                    tc.tile_pool(name="attn", bufs=8) as attnp,
                    tc.tile_pool(name="smalls", bufs=3) as smalls,
                    tc.tile_pool(name="ps_s", bufs=2, space="PSUM") as ps_s,
                    tc.tile_pool(name="ps_o", bufs=2, space="PSUM") as ps_o,
                    tc.tile_pool(name="ps_x", bufs=2, space="PSUM") as ps_x,
                ):
                    # projection pieces fed one-per-head-iteration into the
                    # attention stream.  vB first (needed by AV of head 8 at
                    # iter 9), kT pairs 4-7 spliced to land before their
                    # scores (head 2p at iter 2p).
                    def vb_piece(tt):
                        def run():
                            psv = ps_x.tile([128, QR], F32, tag="x", name="psvb")
                            for c in range(NCT):
                                nc.tensor.matmul(
                                    psv[:],
                                    xT[:, c, tt * 128:(tt + 1) * 128],
                                    wvB[:, c, :],
                                    start=(c == 0), stop=(c == NCT - 1))
                            nc.vector.tensor_copy(
                                vext[:, tt, 8:16, 0:64],
                                psv[:].rearrange("p (h d) -> p h d", h=8))
                        return run

                    def kt_piece(p, u):
                        def run():
                            psk = ps_x.tile([128, QR], F32, tag="x", name="pskb")
                            for c in range(NCT):
                                nc.tensor.matmul(
                                    psk[:],
                                    wkB[:, c, (p - 4) * 128:(p - 3) * 128],
                                    xT[:, c, u * QR:(u + 1) * QR],
                                    start=(c == 0), stop=(c == NCT - 1))
                            nc.scalar.activation(kT[:, p, u * QR:(u + 1) * QR], psk[:], Copy)
                        return run

                    # per-iteration piece lists.  Deadlines: kT pair p must be
                    # fully written before iteration 2p's scores (slot <=
                    # 2p-1); vB before iteration 9's AV of head 8.
                    pieces = [[] for _ in range(H)]
                    pieces[0] = [vb_piece(0), vb_piece(1)]
                    pieces[1] = [vb_piece(2), vb_piece(3)]
                    pieces[2] = [vb_piece(4)]
                    pieces[3] = [vb_piece(5)]
                    pieces[4] = [vb_piece(6)]
                    pieces[5] = [vb_piece(7)]
                    pieces[6] = [kt_piece(4, 0)]
                    pieces[7] = [kt_piece(4, 1)]
                    pieces[8] = [kt_piece(5, 0)]
                    pieces[9] = [kt_piece(5, 1)]
                    pieces[10] = [kt_piece(6, 0)]
                    pieces[11] = [kt_piece(6, 1)]
                    pieces[12] = [kt_piece(7, 0)]
                    pieces[13] = [kt_piece(7, 1)]

                    tiles = {}  # h -> list of (level, at tile)
                    oTs = {}  # h -> oT psum
                    recbs = {}  # h -> broadcast 1/sumexp tile

                    def stage_s(h, js):
                        p, po = h // 2, (h % 2) * 64
                        lst = tiles.setdefault(h, [])
                        for j in js:
                            st = 128 * j
                            sps = ps_s.tile([128, 2, 512], F32, tag="s")
                            for sub in range(2):
                                kb = j + 4 * sub
                                nc.tensor.matmul(
                                    sps[:, sub, st:],
                                    kT[po:po + 64, p, kb * 128:(kb + 1) * 128],
                                    qT[po:po + 64, p, st:],
                                    start=True,
                                    stop=True,
                                )
                            at = attnp.tile([128, 2, 512], BF16, tag="at")
                            nc.scalar.activation(at[:, :, st:], sps[:, :, st:], Exp, scale=0.125)
                            eng = nc.vector if j % 2 == 0 else nc.gpsimd
                            eng.tensor_mul(
                                at[:, :, st:st + 128], at[:, :, st:st + 128], m2[:])
                            lst.append((j, at))

                    def stage_a(h):
                        oT = ps_o.tile([65, QR], F32, tag="o")
                        for j, at in tiles.pop(h):
                            st = 128 * j
                            for sub in range(2):
                                kb = j + 4 * sub
                                nc.tensor.matmul(
                                    oT[:, st:],
                                    vext[:, kb, h, :],
                                    at[:, sub, st:],
                                    start=(j == 0 and sub == 0),
                                    stop=(j == NL - 1 and sub == 1),
                                    skip_group_check=True,
                                )
                        rec = smalls.tile([1, QR], F32, tag="rec")
                        if h < H - 3:
                            nc.vector.reciprocal(rec[:], oT[64:65, :])
                        else:
                            with nc.allow_low_precision(reason="fp32r relabel of fp32 reciprocal"):
                                nc.vector.reciprocal(r(rec[:]), oT[64:65, :])
                        # copy oT out of PSUM immediately so the bank frees
                        # without waiting for the broadcast round trip
                        oTc = smalls.tile([64, QR], BF16, tag="oTc")
                        with nc.allow_low_precision(reason="attention output rounded to bf16"):
                            nc.vector.tensor_copy(oTc[:], oT[0:64, :])
                        oTs[h] = oTc
                        if h < H - 3:
                            # bounce through DRAM to broadcast across 64 rows
                            nc.sync.dma_start(recscr_d[h:h + 1, :], rec[:])
                            recb = smalls.tile([64, QR], F32, tag="recb")
                            nc.sync.dma_start(
                                recb[:], recscr_d[h:h + 1, :].partition_broadcast(64))
                            recbs[h] = recb
                        else:
                            # tail heads: PE rank-1 broadcast avoids the DMA
                            # round-trip latency on the critical path
                            recbs[h] = rec

                    def stage_n(h):
                        p, po = h // 2, (h % 2) * 64
                        oTc = oTs.pop(h)
                        recb = recbs.pop(h)
                        if h >= H - 3:
                            bcf = ps_x.tile([128, QR], F32, tag="x", name="bc")
                            nc.tensor.matmul(bcf[0:64, :], r(ones1f[:]), r(recb[:]),
                                             start=True, stop=True)
                            recb = bcf[0:64, :]
                        with nc.allow_low_precision(reason="attention output rounded to bf16"):
                            nc.vector.tensor_mul(proj_in[po:po + 64, p, :], oTc[:], recb[:])

                    for h in range(H):
                        stage_s(h, (0, 1))
                        for piece in pieces[h]:
                            piece()
                        stage_s(h, (2, 3))
                        if h >= 1:
                            stage_a(h - 1)
                        if h >= 2:
                            stage_n(h - 2)
                    # tight drain: the last head's AV/normalize gates the
                    # output projection and the final 2MB DMA
                    stage_a(H - 1)
                    stage_n(H - 2)
                    stage_n(H - 1)

            # ---------------- output projection ----------------
            with (
                tc.tile_pool(name="fin", bufs=2) as finp,
                tc.tile_pool(name="ps_f", bufs=2, space="PSUM") as ps_f,
            ):
                # row-blocks finalize serially: mul(15) already landed during
                # the attention drain, so each block's output DMA hides under
                # the next block's matmul chain; only m3's DMA is exposed.
                for m in range(QR // 128):
                    psf = ps_f.tile([128, C], F32, tag="f")
                    for p in range(NP):
                        lhs = proj_in[:, p, m * 128:(m + 1) * 128]
                        nc.tensor.matmul(psf[:, 0:512], lhs, woT[:, p, 0:512],
                                         start=(p == 0), stop=False,
                                         skip_group_check=True)
                        nc.tensor.matmul(psf[:, 512:1024], lhs, woT[:, p, 512:1024],
                                         start=(p == 0), stop=False,
                                         skip_group_check=True)
                    # bias via rank-1 accumulate: psf += 1 (x) bo
                    nc.tensor.matmul(psf[:, 0:512], wlhs[:], bo1[0:1, 0:512],
                                     start=False, stop=True, skip_group_check=True)
                    nc.tensor.matmul(psf[:, 512:1024], wlhs[:], bo1[0:1, 512:1024],
                                     start=False, stop=True, skip_group_check=True)
                    fin = finp.tile([128, C], F32, tag="fin")
                    # halves on alternating engines so the copies drain 2x
                    for u in range(2):
                        half = slice(u * 512, (u + 1) * 512)
                        if u == 0:
                            nc.scalar.activation(fin[:, half], psf[:, half], Copy)
                        else:
                            nc.vector.tensor_copy(fin[:, half], psf[:, half])
                        nc.sync.dma_start(
                            out_d[m * 128:(m + 1) * 128, half], fin[:, half])

    nc.compile()
    return nc


def get_nc():
    if "nc" not in _CACHE:
        _CACHE["nc"] = _build()
    return _CACHE["nc"]


def make_in_maps(x, Wq, Wk, Wv, Wo, bo):
    import ml_dtypes

    BF = ml_dtypes.bfloat16
    x = np.asarray(x, dtype=np.float32)
    wq = np.asarray(Wq, np.float32).transpose(1, 0, 2).reshape(C, H * D).astype(BF)
    wk = np.asarray(Wk, np.float32).transpose(1, 0, 2).reshape(C, H * D).astype(BF)
    wv = np.asarray(Wv, np.float32).transpose(1, 0, 2).reshape(C, H * D).astype(BF)
    wqA, wqB = (np.ascontiguousarray(wq[:, :QR]), np.ascontiguousarray(wq[:, QR:]))
    wkA, wkB = (np.ascontiguousarray(wk[:, :QR]), np.ascontiguousarray(wk[:, QR:]))
    wvA, wvB = (np.ascontiguousarray(wv[:, :QR]), np.ascontiguousarray(wv[:, QR:]))
    woT = np.ascontiguousarray(np.asarray(Wo, np.float32).T.astype(BF))
    bo1 = np.ascontiguousarray(np.asarray(bo, np.float32).reshape(1, C).astype(BF))
    k_ = np.arange(128)[:, None]
    i_ = np.arange(128)[None, :]
    m_own = (k_ <= i_).astype(BF)  # own-parity boundary block: k <= q
    in_maps = []
    for core in range(N_CORES):
        b, par = core // 2, core % 2
        perm = np.concatenate([np.arange(par, T, 2), np.arange(1 - par, T, 2)])
        xTp = np.ascontiguousarray(x[b].T[:, perm].astype(BF))
        # other-parity keys 2k+(1-par) vs queries 2i+par: strict for par=0
        m_oth = ((k_ < i_) if par == 0 else (k_ <= i_)).astype(BF)
        m2 = np.ascontiguousarray(np.stack([m_own, m_oth], axis=1))
        in_maps.append({
            "xT": xTp, "wqA": wqA, "wqB": wqB, "wkA": wkA, "wkB": wkB,
            "wvA": wvA, "wvB": wvB, "woT": woT, "bo1": bo1, "m2": m2,
        })
    return in_maps


def kernel(x, Wq, Wk, Wv, Wo, bo):
    from concourse.bass_utils import run_bass_kernel_spmd

    nc = get_nc()
    in_maps = make_in_maps(x, Wq, Wk, Wv, Wo, bo)
    res = run_bass_kernel_spmd(nc, in_maps, list(range(N_CORES)))
    out = np.empty((B, T, C), np.float32)
    for core in range(N_CORES):
        b, par = core // 2, core % 2
        out[b, par::2, :] = res.results[core]["out"]
    return out
